# revision 2
# baseline (speedup 1.0000x reference)
"""CrossViewAttention Trainium2 Bass kernel.

Problem: q,kv [V=6,B=2,C=512,H=32,W=32]; per fused batch (12 total):
  kvp = kv_flat @ W_kv + b_kv -> k,v; 8-head attention(q, k, v); out = y @ W_proj + b_proj.

Layout trick: input (v b c h w) is ALREADY feature-major [C, T=H*W] per batch,
i.e. q^T / kv^T.  The whole pipeline runs in transposed space:
  k^T[c2,t]   = sum_c W_kv[c,c2] kv^T[c,t]          (lhsT=W_kv block, rhs=kv^T)
  v[t,d]      = sum_c kv^T[c,t] W_kv[c,512+d]        (lhsT=kv^T block, rhs=W_kv cols)
  S^T[k,q]    = sum_d kh^T[d,k] qh^T[d,q]            (lhsT=k^T slice, rhs=q^T slice)
  P^T         = exp(S^T/8)            (softmax over k = partition dim; no max-sub,
                                       scores bounded; denom via ones-column below)
  y'^T[d',q]  = sum_k [vh|1][k,d'] P^T[k,q]          (row 64 = softmax denominator)
  y^T         = y'^T[0:64] * recip(y'^T[64])
  out^T[c2,t] = sum_c W_proj[c,c2] y^T[c,t] + b_proj (native output layout)

Sharding (8 cores, no collectives): core c gets full batch c, plus half of the
query dim (half = c%2) of batch 8 + c//2 (kv-proj for split batches is
duplicated on both cores of a pair; attention/out-proj are halved).
"""

import numpy as np

V, BS = 6, 2
NB = V * BS          # 12 fused batches
C = 512
T = 1024
NH = 8
HD = 64
C2 = 2 * C
P = 128
NCORES = 8
TQ = 512             # q-chunk / matmul moving free dim

_CACHE = {}


def _build_nc():
    from contextlib import ExitStack

    from concourse import bacc, mybir, tile

    F32 = mybir.dt.float32
    F32R = mybir.dt.float32r
    EXP = mybir.ActivationFunctionType.Exp
    ADD = mybir.AluOpType.add
    MUL = mybir.AluOpType.mult

    nc = bacc.Bacc("TRN2", target_bir_lowering=False, debug=False,
                   enable_asserts=True, num_devices=NCORES)

    qA = nc.dram_tensor("qA", [C, T], F32, kind="ExternalInput").ap()
    kvA = nc.dram_tensor("kvA", [C, T], F32, kind="ExternalInput").ap()
    qB = nc.dram_tensor("qB", [C, TQ], F32, kind="ExternalInput").ap()
    kvB = nc.dram_tensor("kvB", [C, T], F32, kind="ExternalInput").ap()
    wkv = nc.dram_tensor("wkv", [C, C2], F32, kind="ExternalInput").ap()
    bkv = nc.dram_tensor("bkv", [C2], F32, kind="ExternalInput").ap()
    wpr = nc.dram_tensor("wpr", [C, C], F32, kind="ExternalInput").ap()
    bpr = nc.dram_tensor("bpr", [C], F32, kind="ExternalInput").ap()
    outA = nc.dram_tensor("outA", [C, T], F32, kind="ExternalOutput").ap()
    outB = nc.dram_tensor("outB", [C, TQ], F32, kind="ExternalOutput").ap()

    with tile.TileContext(nc) as tc, ExitStack() as ctx:
        consts = ctx.enter_context(tc.tile_pool(name="consts", bufs=1))
        qpool_a = ctx.enter_context(tc.tile_pool(name="qpa", bufs=1))
        qpool_b = ctx.enter_context(tc.tile_pool(name="qpb", bufs=1))
        kvpool = ctx.enter_context(tc.tile_pool(name="kvp", bufs=2))
        ktpool = ctx.enter_context(tc.tile_pool(name="ktp", bufs=1))
        vpool = ctx.enter_context(tc.tile_pool(name="vp", bufs=1))
        ptpool = ctx.enter_context(tc.tile_pool(name="ptp", bufs=2))
        ytpool = ctx.enter_context(tc.tile_pool(name="ytp", bufs=1))
        rcpool = ctx.enter_context(tc.tile_pool(name="rcp", bufs=2))
        rbpool = ctx.enter_context(tc.tile_pool(name="rbp", bufs=2))
        outpool = ctx.enter_context(tc.tile_pool(name="op", bufs=3))
        psum_kv = ctx.enter_context(tc.tile_pool(name="pskv", bufs=2, space="PSUM"))
        psum_s = ctx.enter_context(tc.tile_pool(name="pss", bufs=3, space="PSUM"))
        psum_y = ctx.enter_context(tc.tile_pool(name="psy", bufs=2, space="PSUM"))

        # ---- constants ----
        wkv_sb = consts.tile([P, 4, C2], F32R, tag="wkv")
        nc.gpsimd.dma_start(out=wkv_sb, in_=wkv.rearrange("(b p) n -> p b n", p=P))
        wpr_sb = consts.tile([P, 4, C], F32R, tag="wpr")
        nc.gpsimd.dma_start(out=wpr_sb, in_=wpr.rearrange("(b p) n -> p b n", p=P))
        bk_sb = consts.tile([P, 4], F32, tag="bk")
        nc.sync.dma_start(out=bk_sb, in_=bkv[0:C].rearrange("(b p) -> p b", p=P))
        bp_sb = consts.tile([P, 4], F32, tag="bp")
        nc.sync.dma_start(out=bp_sb, in_=bpr.rearrange("(b p) -> p b", p=P))
        bv_bc = consts.tile([P, C], F32, tag="bv")
        nc.gpsimd.dma_start(out=bv_bc, in_=bkv[None, None, C:C2].broadcast_to([1, P, C]))

        # ---- per-batch inputs ----
        qa_sb = qpool_a.tile([P, 4, T], F32R, tag="qa")
        nc.gpsimd.dma_start(out=qa_sb, in_=qA.rearrange("(b p) t -> p b t", p=P))
        qb_sb = qpool_b.tile([P, 4, TQ], F32R, tag="qb")
        nc.gpsimd.dma_start(out=qb_sb, in_=qB.rearrange("(b p) t -> p b t", p=P))

        for q_sb, kv_dram, nq, out_dram in ((qa_sb, kvA, 2, outA),
                                            (qb_sb, kvB, 1, outB)):
            kv_sb = kvpool.tile([P, 4, T], F32R, tag="kv")
            nc.gpsimd.dma_start(out=kv_sb, in_=kv_dram.rearrange("(b p) t -> p b t", p=P))

            # ---- kv projection: k^T (feature-major) ----
            kT = ktpool.tile([P, 4, T], F32R, tag="kT")
            for c2b in range(4):
                for tch in range(2):
                    pk = psum_kv.tile([P, TQ], F32, tag="pkv")
                    for cb in range(4):
                        nc.tensor.matmul(
                            pk,
                            lhsT=wkv_sb[:, cb, c2b * P:(c2b + 1) * P],
                            rhs=kv_sb[:, cb, tch * TQ:(tch + 1) * TQ],
                            start=cb == 0, stop=cb == 3)
                    nc.vector.tensor_tensor(
                        out=kT[:, c2b, tch * TQ:(tch + 1) * TQ], in0=pk,
                        in1=bk_sb[:, c2b:c2b + 1].broadcast_to([P, TQ]), op=ADD)

            # ---- kv projection: v (T-major) + ones column ----
            vsb = vpool.tile([P, 8, NH, HD + 1], F32R, tag="v")
            nc.vector.tensor_scalar(
                out=vsb[:, :, :, HD],
                in0=bv_bc[:, 0:HD].rearrange("p (a b) -> p a b", a=8),
                scalar1=0.0, scalar2=1.0, op0=MUL, op1=ADD)
            for tb in range(8):
                pv = psum_kv.tile([P, C], F32, tag="pkv")
                for cb in range(4):
                    nc.tensor.matmul(
                        pv,
                        lhsT=kv_sb[:, cb, tb * P:(tb + 1) * P],
                        rhs=wkv_sb[:, cb, C:C2],
                        start=cb == 0, stop=cb == 3)
                nc.vector.tensor_tensor(
                    out=vsb[:, tb, :, 0:HD],
                    in0=pv.rearrange("p (h d) -> p h d", h=NH),
                    in1=bv_bc.rearrange("p (h d) -> p h d", h=NH), op=ADD)

            # ---- attention ----
            yT = ytpool.tile([P, 4, T], F32R, tag="yT")
            for h in range(NH):
                pb, po = h // 2, (h % 2) * HD
                for qc in range(nq):
                    pt = ptpool.tile([P, 8, TQ], F32R, tag="pt")
                    for kb in range(8):
                        ps_s = psum_s.tile([P, TQ], F32, tag="ps")
                        nc.tensor.matmul(
                            ps_s,
                            lhsT=kT[po:po + HD, pb, kb * P:(kb + 1) * P],
                            rhs=q_sb[po:po + HD, pb, qc * TQ:(qc + 1) * TQ],
                            start=True, stop=True)
                        nc.scalar.activation(out=pt[:, kb, :], in_=ps_s,
                                             func=EXP, scale=0.125)
                    py = psum_y.tile([HD + 1, TQ], F32, tag="py")
                    for kb in range(8):
                        nc.tensor.matmul(
                            py,
                            lhsT=vsb[:, kb, h, :],
                            rhs=pt[:, kb, :],
                            start=kb == 0, stop=kb == 7)
                    rc = rcpool.tile([1, TQ], F32, tag="rc")
                    nc.vector.reciprocal(rc, py[HD:HD + 1, :])
                    rb = rbpool.tile([HD, TQ], F32, tag="rb")
                    nc.gpsimd.dma_start(
                        out=rb, in_=rc[0:1, None, :].broadcast_to([1, HD, TQ]))
                    nc.vector.tensor_tensor(
                        out=yT[po:po + HD, pb, qc * TQ:(qc + 1) * TQ],
                        in0=py[0:HD, :], in1=rb, op=MUL)

            # ---- output projection ----
            for c2b in range(4):
                for tch in range(nq):
                    pk = psum_kv.tile([P, TQ], F32, tag="pkv")
                    for cb in range(4):
                        nc.tensor.matmul(
                            pk,
                            lhsT=wpr_sb[:, cb, c2b * P:(c2b + 1) * P],
                            rhs=yT[:, cb, tch * TQ:(tch + 1) * TQ],
                            start=cb == 0, stop=cb == 3)
                    ot = outpool.tile([P, TQ], F32, tag="ot")
                    nc.vector.tensor_tensor(
                        out=ot, in0=pk,
                        in1=bp_sb[:, c2b:c2b + 1].broadcast_to([P, TQ]), op=ADD)
                    nc.sync.dma_start(
                        out=out_dram[c2b * P:(c2b + 1) * P, tch * TQ:(tch + 1) * TQ],
                        in_=ot)

    nc.compile()
    return nc


def get_nc():
    if "nc" not in _CACHE:
        _CACHE["nc"] = _build_nc()
    return _CACHE["nc"]


def make_in_maps(q, kv, W_kv, b_kv, W_proj, b_proj):
    qf = np.ascontiguousarray(q.reshape(NB, C, T))
    kvf = np.ascontiguousarray(kv.reshape(NB, C, T))
    in_maps = []
    for c in range(NCORES):
        bA, bB, half = c, 8 + c // 2, c % 2
        in_maps.append({
            "qA": qf[bA],
            "kvA": kvf[bA],
            "qB": np.ascontiguousarray(qf[bB][:, half * TQ:(half + 1) * TQ]),
            "kvB": kvf[bB],
            "wkv": np.ascontiguousarray(W_kv),
            "bkv": np.ascontiguousarray(b_kv),
            "wpr": np.ascontiguousarray(W_proj),
            "bpr": np.ascontiguousarray(b_proj),
        })
    return in_maps


def assemble_out(results):
    out = np.empty((NB, C, T), np.float32)
    for c in range(NCORES):
        bB, half = 8 + c // 2, c % 2
        out[c] = results[c]["outA"]
        out[bB][:, half * TQ:(half + 1) * TQ] = results[c]["outB"]
    return out.reshape(V, BS, C, 32, 32)


def kernel(**inputs):
    from concourse.bass_utils import run_bass_kernel_spmd

    nc = get_nc()
    in_maps = make_in_maps(inputs["q"], inputs["kv"], inputs["W_kv"],
                           inputs["b_kv"], inputs["W_proj"], inputs["b_proj"])
    res = run_bass_kernel_spmd(nc, in_maps, core_ids=list(range(NCORES)))
    return assemble_out(res.results)
